# revision 14
# baseline (speedup 1.0000x reference)
"""Trainium2 Bass kernel for a transformer encoder layer (B=4, S=2048,
D=1024, H=16, DFF=4096, fp32).

Sharding: 8 cores = (batch b, query-half). Each core computes K/V for its
full batch (replicated within the pair) and Q/attention/FFN for its 1024
query tokens. No collectives.

On-chip dataflow (per core, all layouts chosen so every matmul contracts
along the partition dim with zero on-chip transposes except out1):
  xT [D, S] (host-transposed, bf16)
  QT = wq.T-chunks @ xT   -> [128, 8, 1024]  bf16 (depth-major, head pairs)
  KT -> [128, 8, 2048] bf16 ; V -> [128, 16tok, 16head, 65] bf16 (col 64 = ones)
  S^T[k,q] = KT-slice.T @ QT-slice ; exp on ACT (mask+1/8 scale fused) -> PT bf16
  ctxT+denom = V_aug.T @ PT (ones column gives softmax denominator for free)
  attn_out = ctxT.T @ wo (f32r) ; +x (+bo host-folded) ; LN1
  PE-transpose out1 -> out1T ; FFN1 = relu(w1.T @ out1T + b1) -> hT f32
  FFN2 = hT.T @ w2 (+b2 via K=1 ones-row matmul) ; +out1 ; LN2 -> out
"""

import sys

sys.path.insert(0, "/opt/trn_rl_repo")

import numpy as np
import ml_dtypes

import concourse.bass as bass
import concourse.tile as tile
from concourse import bacc, mybir
from concourse.bass_utils import run_bass_kernel_spmd
from concourse.masks import make_identity

P = 128
D = 1024
S = 2048
TQ = 1024  # query tokens per core
H = 16
DEP = 64
DFF = 4096
F32 = mybir.dt.float32
F32R = mybir.dt.float32r
BF16 = mybir.dt.bfloat16
AF = mybir.ActivationFunctionType
ALU = mybir.AluOpType
EPS = 1e-6


def build():
    nc = bacc.Bacc("TRN2", target_bir_lowering=False)

    # ---- DRAM I/O ----
    xt = nc.dram_tensor("xt", [D, S], BF16, kind="ExternalInput")
    xq = nc.dram_tensor("xq", [TQ, D], F32, kind="ExternalInput")  # +bo folded
    maskb = nc.dram_tensor("maskb", [S], F32, kind="ExternalInput")
    wq = nc.dram_tensor("wq", [D, D], BF16, kind="ExternalInput")
    wk = nc.dram_tensor("wk", [D, D], BF16, kind="ExternalInput")
    wv = nc.dram_tensor("wv", [D, D], BF16, kind="ExternalInput")
    bq = nc.dram_tensor("bq", [D], F32, kind="ExternalInput")
    bk = nc.dram_tensor("bk", [D], F32, kind="ExternalInput")
    bv = nc.dram_tensor("bv", [D], F32, kind="ExternalInput")
    wo = nc.dram_tensor("wo", [D, D], F32R, kind="ExternalInput")
    w1 = nc.dram_tensor("w1", [D, DFF], F32R, kind="ExternalInput")
    b1 = nc.dram_tensor("b1", [DFF], F32, kind="ExternalInput")
    w2 = nc.dram_tensor("w2", [DFF, D], F32R, kind="ExternalInput")
    b2 = nc.dram_tensor("b2", [D], F32R, kind="ExternalInput")
    g1 = nc.dram_tensor("g1", [D], F32, kind="ExternalInput")
    be1 = nc.dram_tensor("be1", [D], F32, kind="ExternalInput")
    g2 = nc.dram_tensor("g2", [D], F32, kind="ExternalInput")
    be2 = nc.dram_tensor("be2", [D], F32, kind="ExternalInput")
    out = nc.dram_tensor("out", [TQ, D], F32, kind="ExternalOutput")

    def bcast(handle, n):
        # [n]-vector broadcast to all 128 partitions via stride-0 DMA
        return bass.AP(tensor=handle, offset=0, ap=[[0, P], [1, n]])

    with tile.TileContext(nc) as tc:
        consts = tc.alloc_tile_pool(name="consts", bufs=1)

        mask_bias = consts.tile([P, S // P], F32)  # mask[k]*-1e9, k = kc*128+p
        mraw = consts.tile([P, S // P], F32, tag="mraw")
        nc.sync.dma_start(out=mraw, in_=maskb.ap().rearrange("(kc p) -> p kc", p=P))
        nc.vector.tensor_scalar_mul(out=mask_bias, in0=mraw, scalar1=-1e9)

        bq_sb = consts.tile([P, D // P], F32)
        bk_sb = consts.tile([P, D // P], F32)
        nc.sync.dma_start(out=bq_sb, in_=bq.ap().rearrange("(m p) -> p m", p=P))
        nc.sync.dma_start(out=bk_sb, in_=bk.ap().rearrange("(m p) -> p m", p=P))
        bv_bc = consts.tile([P, D], F32)
        nc.sync.dma_start(out=bv_bc, in_=bcast(bv, D))
        b1_sb = consts.tile([P, DFF // P], F32)
        nc.sync.dma_start(out=b1_sb, in_=b1.ap().rearrange("(m p) -> p m", p=P))
        b2_sb = consts.tile([1, D], F32R)
        nc.sync.dma_start(out=b2_sb, in_=bass.AP(tensor=b2, offset=0, ap=[[0, 1], [1, D]]))
        ones_row = consts.tile([1, P], F32R)
        ones_f32 = consts.tile([1, P], F32, tag="ones_f32")
        nc.vector.memset(ones_f32, 1.0)
        nc.vector.tensor_copy(out=ones_row, in_=ones_f32)
        g1_bc = consts.tile([P, D], F32)
        be1_bc = consts.tile([P, D], F32)
        g2_bc = consts.tile([P, D], F32)
        be2_bc = consts.tile([P, D], F32)
        nc.sync.dma_start(out=g1_bc, in_=bcast(g1, D))
        nc.sync.dma_start(out=be1_bc, in_=bcast(be1, D))
        nc.sync.dma_start(out=g2_bc, in_=bcast(g2, D))
        nc.sync.dma_start(out=be2_bc, in_=bcast(be2, D))
        ident = consts.tile([P, P], F32)
        make_identity(nc, ident)
        eps_t = consts.tile([P, 1], F32)
        nc.vector.memset(eps_t, EPS)

        # ---------- persistent activation tensors ----------
        qkv_pool = tc.alloc_tile_pool(name="qkv", bufs=1)
        # depth-major Q/K: partition = (h%2)*64 + d, plane = h//2
        QT = qkv_pool.tile([P, D // P, TQ], BF16)
        KT = qkv_pool.tile([P, D // P, S], BF16)
        # V: [tok-part, tok-chunk, head, 65]; col 64 = ones (softmax denom)
        V = qkv_pool.tile([P, S // P, H, DEP + 1], BF16)

        # ================= Phase A: QKV projections =================
        with tc.tile_pool(name="phA", bufs=1) as pa, \
             tc.tile_pool(name="phA_w", bufs=3) as paw, \
             tc.tile_pool(name="psA", bufs=4, space="PSUM") as psA:
            xt_sb = pa.tile([P, D // P, S], BF16)
            xt_r = xt.ap().rearrange("(ko p) t -> p ko t", p=P)
            for ki in range(D // P):
                nc.sync.dma_start(out=xt_sb[:, ki], in_=xt_r[:, ki])

            # ones columns of V
            for t in range(S // P):
                nc.gpsimd.memset(V[:, t, :, DEP:DEP + 1], 1.0)

            # QT and KT: lhsT = w chunk [128din, 128dout], rhs = xT.
            # Host reorders tokens so this core's queries are tokens 0..TQ-1.
            for w_h, b_sb, Tt, ntok in (
                (wq, bq_sb, QT, TQ),
                (wk, bk_sb, KT, S),
            ):
                w_r = w_h.ap().rearrange("(ko p) n -> p ko n", p=P)
                for m in range(D // P):
                    wt = paw.tile([P, D // P, P], BF16, tag="wqk")
                    nc.sync.dma_start(out=wt, in_=w_r[:, :, m * P:(m + 1) * P])
                    for qc in range(ntok // 512):
                        ps = psA.tile([P, 512], F32)
                        for ki in range(D // P):
                            src = xt_sb[:, ki, qc * 512:(qc + 1) * 512]
                            nc.tensor.matmul(ps, wt[:, ki], src,
                                             start=(ki == 0), stop=(ki == D // P - 1))
                        nc.scalar.activation(
                            out=Tt[:, m, qc * 512:(qc + 1) * 512], in_=ps,
                            func=AF.Identity, bias=b_sb[:, m:m + 1])

            # V: lhsT = xT token-chunk, rhs = wv columns
            wv_r = wv.ap().rearrange("(ko p) n -> p ko n", p=P)
            for n in range(2):
                wvt = paw.tile([P, D // P, 512], BF16, tag="wv")
                nc.sync.dma_start(out=wvt, in_=wv_r[:, :, n * 512:(n + 1) * 512])
                for t in range(S // P):
                    ps = psA.tile([P, 512], F32)
                    for ki in range(D // P):
                        nc.tensor.matmul(ps, xt_sb[:, ki, t * P:(t + 1) * P],
                                         wvt[:, ki],
                                         start=(ki == 0), stop=(ki == D // P - 1))
                    nc.vector.scalar_tensor_tensor(
                        out=V[:, t, n * 8:(n + 1) * 8, 0:DEP],
                        in0=ps.rearrange("p (h d) -> p h d", h=8),
                        scalar=1.0, op0=ALU.bypass, op1=ALU.add,
                        in1=bv_bc[:, n * 512:(n + 1) * 512].rearrange(
                            "p (h d) -> p h d", h=8))

        # ================= Phase B: attention =================
        ctx_pool = tc.alloc_tile_pool(name="ctx", bufs=1, side="right")
        ctxT = ctx_pool.tile([P, D // P, TQ], F32R)  # head h at part (h%2)*64, plane h//2
        denom = ctx_pool.tile([H, TQ], F32)
        recip = ctx_pool.tile([H, TQ], F32)
        recip_dram = nc.dram_tensor("recip_scratch", [H, TQ], F32, kind="Internal")

        with tc.tile_pool(name="phB_pt", bufs=1) as ptp, \
             tc.tile_pool(name="phB_st", bufs=2) as stst, \
             tc.tile_pool(name="phB_misc", bufs=2) as pbm, \
             tc.tile_pool(name="psB_st", bufs=2, space="PSUM") as psST, \
             tc.tile_pool(name="psB_ctx", bufs=2, space="PSUM") as psCTX:
            for hp in range(H // 2):
                for qc in range(TQ // 512):
                    # PT[k-part, kc, head(e/o), q] for this (hp, qc)
                    pt = ptp.tile([P, S // P, 2, 512], BF16, tag="pt")
                    for kc in range(S // P):
                        st = psST.tile([P, 2, 512], F32, tag="st")
                        for e in range(2):
                            nc.tensor.matmul(
                                st[:, e],
                                KT[e * DEP:(e + 1) * DEP, hp, kc * P:(kc + 1) * P],
                                QT[e * DEP:(e + 1) * DEP, hp, qc * 512:(qc + 1) * 512],
                                start=True, stop=True)
                        nc.scalar.activation(
                            out=pt[:, kc], in_=st, func=AF.Exp,
                            bias=mask_bias[:, kc:kc + 1], scale=0.125)
                    for e in range(2):
                        h = hp * 2 + e
                        cps = psCTX.tile([P, 512], F32, tag="cps")
                        for kc in range(S // P):
                            nc.tensor.matmul(
                                cps[0:DEP + 1], V[:, kc, h, :], pt[:, kc, e],
                                start=(kc == 0), stop=(kc == S // P - 1))
                        qsl = slice(qc * 512, (qc + 1) * 512)
                        if e == 0:
                            nc.vector.tensor_copy(out=ctxT[0:DEP, hp, qsl],
                                                  in_=cps[0:DEP])
                        else:
                            stg = pbm.tile([DEP, 512], F32R, tag="stg")
                            nc.vector.tensor_copy(out=stg, in_=cps[0:DEP])
                            nc.sync.dma_start(out=ctxT[DEP:P, hp, qsl], in_=stg)
                        dst = pbm.tile([P, 512], F32, tag="dst")
                        nc.vector.tensor_copy(out=dst[DEP:DEP + 1], in_=cps[DEP:DEP + 1])
                        nc.sync.dma_start(out=denom[h:h + 1, qsl],
                                          in_=dst[DEP:DEP + 1])

            # softmax normalization: ctxT *= 1/denom (per head, per q)
            nc.vector.reciprocal_approx_fast(out=recip, in_=denom)
            nc.sync.dma_start(out=recip_dram.ap(), in_=recip)
            for h in range(H):
                for qc in range(TQ // 512):
                    rb = pbm.tile([P, 512], F32, tag="rb")
                    prt = slice((h % 2) * DEP, (h % 2) * DEP + DEP)
                    qsl = slice(qc * 512, (qc + 1) * 512)
                    nc.sync.dma_start(
                        out=rb[prt],
                        in_=recip_dram.ap()[h:h + 1, qsl].partition_broadcast(
                            DEP).squeeze(1))
                    nc.vector.scalar_tensor_tensor(
                        out=ctxT[prt, h // 2, qsl], in0=ctxT[prt, h // 2, qsl],
                        scalar=1.0, op0=ALU.bypass, op1=ALU.mult, in1=rb[prt])

        qkv_pool.release()

        # ================= Phase C: wo + residual + LN1 + transpose ==========
        ffnin = tc.alloc_tile_pool(name="ffnin", bufs=1)
        out1 = ffnin.tile([P, TQ // P, D], F32)   # token qm*128+p
        out1T = ffnin.tile([P, D // P, TQ], F32R)  # d dm*128+p

        with tc.tile_pool(name="phC", bufs=1) as pc, \
             tc.tile_pool(name="phC_s", bufs=4) as pcs, \
             tc.tile_pool(name="psC", bufs=3, space="PSUM") as psC, \
             tc.tile_pool(name="psCT", bufs=2, space="PSUM") as psCT:
            wo_sb = pc.tile([P, D // P, D], F32R)
            wo_r = wo.ap().rearrange("(ko p) n -> p ko n", p=P)
            for ki in range(D // P):
                nc.sync.dma_start(out=wo_sb[:, ki], in_=wo_r[:, ki])
            xq_r = xq.ap().rearrange("(qm p) d -> p qm d", p=P)

            for qm in range(TQ // P):
                xq_t = pcs.tile([P, D], F32, tag="xqt")
                nc.sync.dma_start(out=xq_t, in_=xq_r[:, qm])
                for n in range(2):
                    ps = psC.tile([P, 512], F32)
                    for ki in range(D // P):
                        nc.tensor.matmul(
                            ps, (ctxT[:, ki, qm * P:(qm + 1) * P]),
                            (wo_sb[:, ki, n * 512:(n + 1) * 512]),
                            start=(ki == 0), stop=(ki == D // P - 1))
                    nc.vector.scalar_tensor_tensor(
                        out=out1[:, qm, n * 512:(n + 1) * 512], in0=ps,
                        scalar=1.0, op0=ALU.bypass, op1=ALU.add,
                        in1=xq_t[:, n * 512:(n + 1) * 512])
                # LN1 on out1[:, qm, :] in place
                _layernorm(nc, pcs, out1[:, qm], g1_bc, be1_bc, eps_t)

            # transpose out1 -> out1T
            for qm in range(TQ // P):
                for dm in range(D // P):
                    tp = psCT.tile([P, P], F32, tag="tp")
                    nc.tensor.transpose(tp, out1[:, qm, dm * P:(dm + 1) * P], ident)
                    nc.vector.tensor_copy(
                        out=out1T[:, dm, qm * P:(qm + 1) * P], in_=tp)

        ctx_pool.release()

        # ================= Phase D: FFN + LN2 =================
        with tc.tile_pool(name="phD_h", bufs=1) as pdh, \
             tc.tile_pool(name="phD_w", bufs=3) as pdw, \
             tc.tile_pool(name="phD_s", bufs=4) as pds, \
             tc.tile_pool(name="psD1", bufs=3, space="PSUM") as psD1, \
             tc.tile_pool(name="psD2", bufs=1, space="PSUM") as psD2:
            w1_r = w1.ap().rearrange("(ko p) n -> p ko n", p=P)
            w2_r = w2.ap().rearrange("(ko p) n -> p ko n", p=P)
            for qc in range(TQ // 512):
                hT = pdh.tile([P, DFF // P, 512], F32R, tag="hT")
                for dm in range(DFF // P):
                    w1t = pdw.tile([P, D // P, P], F32R, tag="w1t")
                    nc.sync.dma_start(out=w1t, in_=w1_r[:, :, dm * P:(dm + 1) * P])
                    ps = psD1.tile([P, 512], F32, tag="ps1")
                    for ki in range(D // P):
                        nc.tensor.matmul(
                            ps, (w1t[:, ki]),
                            (out1T[:, ki, qc * 512:(qc + 1) * 512]),
                            start=(ki == 0), stop=(ki == D // P - 1))
                    nc.scalar.activation(out=hT[:, dm], in_=ps, func=AF.Relu,
                                         bias=b1_sb[:, dm:dm + 1])
                for n in range(2):
                    pss = [psD2.tile([P, 512], F32, tag=f"ps2_{i}", name=f"ps2_{i}") for i in range(4)]
                    for ds_ in range(DFF // P):
                        w2t = pdw.tile([P, 512], F32R, tag="w2t")
                        nc.sync.dma_start(
                            out=w2t, in_=w2_r[:, ds_, n * 512:(n + 1) * 512])
                        for j in range(4):
                            nc.tensor.matmul(
                                pss[j], (hT[:, ds_, j * P:(j + 1) * P]),
                                (w2t), start=(ds_ == 0), stop=False)
                    for j in range(4):
                        # += b2 broadcast over tokens (K=1 ones-row matmul)
                        nc.tensor.matmul(pss[j], (ones_row),
                                         (b2_sb[:, n * 512:(n + 1) * 512]),
                                         start=False, stop=True)
                        qm = qc * 4 + j
                        nc.vector.scalar_tensor_tensor(
                            out=out1[:, qm, n * 512:(n + 1) * 512], in0=pss[j],
                            scalar=1.0, op0=ALU.bypass, op1=ALU.add,
                            in1=out1[:, qm, n * 512:(n + 1) * 512])
                for j in range(4):
                    qm = qc * 4 + j
                    _layernorm(nc, pds, out1[:, qm], g2_bc, be2_bc, eps_t)
                    nc.sync.dma_start(
                        out=out.ap().rearrange("(qm p) d -> p qm d", p=P)[:, qm],
                        in_=out1[:, qm])

        ffnin.release()
        consts.release()

    nc.compile()
    return nc


def _layernorm(nc, pool, x_ap, g_bc, be_bc, eps_t):
    """In-place LN over the free dim of x_ap [128, D]."""
    stats = pool.tile([P, D // 512, 6], F32, tag="ln_stats")
    mv = pool.tile([P, 2], F32, tag="ln_mv")
    xg = x_ap.rearrange("p (s f) -> p s f", f=512)
    for s in range(D // 512):
        nc.vector.bn_stats(out=stats[:, s], in_=xg[:, s])
    nc.vector.bn_aggr(out=mv, in_=stats)
    rstd = pool.tile([P, 1], F32, tag="ln_rstd")
    nc.scalar.activation(out=rstd, in_=mv[:, 1:2], func=AF.Sqrt, bias=eps_t)
    nc.vector.reciprocal(out=rstd, in_=rstd)
    nc.vector.tensor_scalar(out=x_ap, in0=x_ap, scalar1=mv[:, 0:1],
                            scalar2=rstd, op0=ALU.subtract, op1=ALU.mult)
    nc.vector.tensor_tensor(out=x_ap, in0=x_ap, in1=g_bc, op=ALU.mult)
    nc.vector.tensor_tensor(out=x_ap, in0=x_ap, in1=be_bc, op=ALU.add)


_NC_CACHE = None


def _get_nc():
    global _NC_CACHE
    if _NC_CACHE is None:
        _NC_CACHE = build()
    return _NC_CACHE


def _prep_in_maps(inputs):
    x = np.asarray(inputs["x"], dtype=np.float32)        # [4, 2048, 1024]
    mask = np.asarray(inputs["mask"], dtype=np.float32)  # [4, 1, 1, 2048]
    bf = ml_dtypes.bfloat16
    w = {k: np.asarray(inputs[k], dtype=np.float32) for k in
         ("wq", "bq", "wk", "bk", "wv", "bv", "wo", "bo", "w1", "b1",
          "w2", "b2", "g1", "beta1", "g2", "beta2")}

    shared = {
        "wq": w["wq"].astype(bf), "wk": w["wk"].astype(bf), "wv": w["wv"].astype(bf),
        "bq": w["bq"], "bk": w["bk"], "bv": w["bv"],
        "wo": w["wo"], "w1": w["w1"], "b1": w["b1"],
        "w2": w["w2"], "b2": w["b2"],
        "g1": w["g1"], "be1": w["beta1"], "g2": w["g2"], "be2": w["beta2"],
    }
    in_maps = []
    for c in range(8):
        b, half = c // 2, c % 2
        m = dict(shared)
        # reorder tokens so this core's queries come first (attention is
        # permutation-invariant over keys; mask is reordered to match)
        xb = x[b]
        order = np.r_[half * TQ:(half + 1) * TQ, (1 - half) * TQ:(2 - half) * TQ]
        m["xt"] = np.ascontiguousarray(xb[order].T).astype(bf)
        m["xq"] = np.ascontiguousarray(
            xb[half * TQ:(half + 1) * TQ] + w["bo"][None, :])
        m["maskb"] = np.ascontiguousarray(mask[b, 0, 0][order])
        in_maps.append(m)
    return in_maps


def kernel(**inputs):
    in_maps = _prep_in_maps(inputs)
    nc = _get_nc()
    res = run_bass_kernel_spmd(nc, in_maps, core_ids=list(range(8)))
    outp = np.empty((4, 2048, 1024), dtype=np.float32)
    for c in range(8):
        b, half = c // 2, c % 2
        outp[b, half * TQ:(half + 1) * TQ] = res.results[c]["out"]
    return outp


# revision 17
# speedup vs baseline: 1.1005x; 1.1005x over previous
"""Trainium2 Bass kernel for a transformer encoder layer (B=4, S=2048,
D=1024, H=16, DFF=4096, fp32).

Sharding: 8 cores = (batch b, query-half). Each core computes K/V for its
full batch (replicated within the pair) and Q/attention/FFN for its 1024
query tokens. No collectives. Host reorders tokens so each core's queries
are tokens 0..1023 of its (permutation-invariant) key set.

Per-core dataflow (layouts chosen so every matmul contracts along the
partition dim; the only on-chip transpose is out1 -> out1T):
  xT [D, S] host-transposed bf16
  QT/KT depth-major bf16 (head pairs packed 64+64 in partitions)
  V [tok, head, 65] bf16 with a ones column -> softmax denominator for free
  S^T = KT.T @ QT per (head, kc); exp on ACT (mask + 1/8 scale fused) -> PT bf16
  ctxT+denom = V_aug.T @ PT ; normalize via 1/denom broadcast ; ctxT bf16
  attn_out = ctxT.T @ wo (bf16) ; +x (+bo host-folded) ; LN1
  PE-transpose out1 -> out1T f32r ; FFN1 = relu(w1.T @ out1T + b1) -> hT f32r
  FFN2 = hT.T @ w2 (+b2 via K=1 ones-row matmul) ; +out1 ; LN2 -> out

Projection and attention emission is interleaved in two halves so the
scalar engine starts softmax exps while the PE is still projecting.
"""

import sys

sys.path.insert(0, "/opt/trn_rl_repo")

import numpy as np
import ml_dtypes

import concourse.bass as bass
import concourse.tile as tile
from concourse import bacc, mybir
from concourse.bass_utils import run_bass_kernel_spmd
from concourse.masks import make_identity

P = 128
D = 1024
S = 2048
TQ = 1024  # query tokens per core
H = 16
DEP = 64
DFF = 4096
F32 = mybir.dt.float32
F32R = mybir.dt.float32r
BF16 = mybir.dt.bfloat16
AF = mybir.ActivationFunctionType
ALU = mybir.AluOpType
EPS = 1e-6


def build():
    nc = bacc.Bacc("TRN2", target_bir_lowering=False)

    # ---- DRAM I/O ----
    xt = nc.dram_tensor("xt", [D, S], BF16, kind="ExternalInput")
    xq = nc.dram_tensor("xq", [TQ, D], F32, kind="ExternalInput")  # +bo folded
    maskb = nc.dram_tensor("maskb", [S], F32, kind="ExternalInput")
    wq = nc.dram_tensor("wq", [D, D], BF16, kind="ExternalInput")
    wk = nc.dram_tensor("wk", [D, D], BF16, kind="ExternalInput")
    wv = nc.dram_tensor("wv", [D, D], BF16, kind="ExternalInput")
    bq = nc.dram_tensor("bq", [D], F32, kind="ExternalInput")
    bk = nc.dram_tensor("bk", [D], F32, kind="ExternalInput")
    bv = nc.dram_tensor("bv", [D], F32, kind="ExternalInput")
    wo = nc.dram_tensor("wo", [D, D], BF16, kind="ExternalInput")
    w1p = nc.dram_tensor("w1p", [DFF // P, P, D // P, P], F32R, kind="ExternalInput")
    b1 = nc.dram_tensor("b1", [DFF], F32, kind="ExternalInput")
    w2 = nc.dram_tensor("w2", [DFF, D], F32R, kind="ExternalInput")
    b2 = nc.dram_tensor("b2", [D], F32R, kind="ExternalInput")
    g1 = nc.dram_tensor("g1", [D], F32, kind="ExternalInput")
    be1 = nc.dram_tensor("be1", [D], F32, kind="ExternalInput")
    g2 = nc.dram_tensor("g2", [D], F32, kind="ExternalInput")
    be2 = nc.dram_tensor("be2", [D], F32, kind="ExternalInput")
    out = nc.dram_tensor("out", [TQ, D], F32, kind="ExternalOutput")
    recip_dram = nc.dram_tensor("recip_scratch", [H, TQ], F32, kind="Internal")
    warm_dram = nc.dram_tensor("warm_scratch", [P, 16], F32, kind="Internal")

    def bcast(handle, n):
        return bass.AP(tensor=handle, offset=0, ap=[[0, P], [1, n]])

    with tile.TileContext(nc) as tc:
        consts = tc.alloc_tile_pool(name="consts", bufs=1)

        mask_bias = consts.tile([P, S // P], F32)  # mask[k]*-1e9, k = kc*128+p
        nc.gpsimd.dma_start(out=mask_bias,
                            in_=maskb.ap().rearrange("(kc p) -> p kc", p=P))
        nc.vector.tensor_scalar_mul(out=mask_bias, in0=mask_bias, scalar1=-1e9)

        bq_sb = consts.tile([P, D // P], F32)
        bk_sb = consts.tile([P, D // P], F32)
        nc.gpsimd.dma_start(out=bq_sb, in_=bq.ap().rearrange("(m p) -> p m", p=P))
        nc.gpsimd.dma_start(out=bk_sb, in_=bk.ap().rearrange("(m p) -> p m", p=P))
        bv_bc = consts.tile([P, D], F32)
        nc.gpsimd.dma_start(out=bv_bc, in_=bcast(bv, D))
        b1_sb = consts.tile([P, DFF // P], F32)
        nc.gpsimd.dma_start(out=b1_sb, in_=b1.ap().rearrange("(m p) -> p m", p=P))
        b2_sb = consts.tile([1, D], F32R)
        nc.gpsimd.dma_start(out=b2_sb, in_=bass.AP(tensor=b2, offset=0, ap=[[0, 1], [1, D]]))
        ones_row = consts.tile([1, P], F32R)
        ones_f32 = consts.tile([1, P], F32, tag="ones_f32")
        nc.vector.memset(ones_f32, 1.0)
        nc.vector.tensor_copy(out=ones_row, in_=ones_f32)
        ident = consts.tile([P, P], F32)
        make_identity(nc, ident)
        eps_t = consts.tile([P, 1], F32)
        nc.vector.memset(eps_t, EPS)

        # ---------- persistent activation tensors (split per half for
        # clean producer/consumer separation in the interleaved schedule) ----
        qkv_pool = tc.alloc_tile_pool(name="qkv", bufs=1)
        # depth-major Q/K: partition = (h%2)*64 + d, plane = h//2 - 4*half
        QTh = [qkv_pool.tile([P, 4, TQ], BF16, name=f"QT{i}") for i in range(2)]
        KTh = [qkv_pool.tile([P, 4, S], BF16, name=f"KT{i}") for i in range(2)]
        # V: [tok-part, tok-chunk, head - 8*half, 65]; col 64 = ones
        Vh = [qkv_pool.tile([P, S // P, 8, DEP + 1], BF16, name=f"V{i}")
              for i in range(2)]

        # ---------- attention output (right stack: outlives qkv) ----------
        ctx_pool = tc.alloc_tile_pool(name="ctx", bufs=1, side="right")
        ctxT = ctx_pool.tile([P, D // P, TQ], BF16)  # head h at part (h%2)*64
        denom = ctx_pool.tile([H, TQ], F32)
        recip = ctx_pool.tile([H, TQ], F32)

        # ---------- PE warm-up: a few fp32 matmuls so HAM un-throttles ----
        with tc.tile_pool(name="warm", bufs=1) as wp, \
             tc.tile_pool(name="warm_ps", bufs=1, space="PSUM") as wps:
            wjunk = wp.tile([P, 16], F32)
            wpt = wps.tile([P, P], F32)
            for i in range(10):
                nc.tensor.matmul(wpt, ident, ident, start=(i == 0),
                                 stop=(i == 9))
            nc.vector.tensor_copy(out=wjunk, in_=wpt[:, 0:16])
            nc.sync.dma_start(out=warm_dram.ap(), in_=wjunk)

        # ================= interleaved projections + attention ============
        with tc.tile_pool(name="phA", bufs=1) as pa, \
             tc.tile_pool(name="phA_wqk", bufs=3) as paw, \
             tc.tile_pool(name="phA_wv", bufs=1) as pawv, \
             tc.tile_pool(name="phB_pt", bufs=1) as ptp, \
             tc.tile_pool(name="phB_misc", bufs=2) as pbm, \
             tc.tile_pool(name="psA", bufs=2, space="PSUM") as psA, \
             tc.tile_pool(name="psB_st", bufs=2, space="PSUM") as psST, \
             tc.tile_pool(name="psB_ctx", bufs=2, space="PSUM") as psCTX:
            xt_r = xt.ap().rearrange("(ko p) t -> p ko t", p=P)
            for half in range(2):
                for t in range(S // P):
                    nc.gpsimd.memset(Vh[half][:, t, :, DEP:DEP + 1], 1.0)

            wq_r = wq.ap().rearrange("(ko p) n -> p ko n", p=P)
            wk_r = wk.ap().rearrange("(ko p) n -> p ko n", p=P)
            wv_r = wv.ap().rearrange("(ko p) n -> p ko n", p=P)

            def project_qk(xt_h, th, w_r, b_sb, Tt, m):
                # lhsT = w chunk [128din, 128dout]; rhs = xT token-half
                wt = paw.tile([P, D // P, P], BF16, tag="wqk", name="wqk")
                nc.sync.dma_start(out=wt, in_=w_r[:, :, m * P:(m + 1) * P])
                for qc in range(2):
                    ps = psA.tile([P, 512], F32, tag="psA", name="psA")
                    for ki in range(D // P):
                        nc.tensor.matmul(ps, wt[:, ki],
                                         xt_h[:, ki, qc * 512:(qc + 1) * 512],
                                         start=(ki == 0), stop=(ki == D // P - 1))
                    nc.vector.tensor_scalar(
                        out=Tt[:, m % 4, th * TQ + qc * 512:
                               th * TQ + (qc + 1) * 512], in0=ps,
                        scalar1=b_sb[:, m:m + 1], scalar2=None, op0=ALU.add)

            def project_v(xt_h, th, ah):
                wvt = pawv.tile([P, D // P, 512], BF16, tag="wv", name="wv")
                nc.sync.dma_start(out=wvt,
                                  in_=wv_r[:, :, ah * 512:(ah + 1) * 512])
                for tl in range(TQ // P):
                    t = th * (TQ // P) + tl
                    ps = psA.tile([P, 512], F32, tag="psA", name="psA")
                    for ki in range(D // P):
                        nc.tensor.matmul(ps, xt_h[:, ki, tl * P:(tl + 1) * P],
                                         wvt[:, ki],
                                         start=(ki == 0), stop=(ki == D // P - 1))
                    nc.vector.scalar_tensor_tensor(
                        out=Vh[ah][:, t, :, 0:DEP],
                        in0=ps.rearrange("p (h d) -> p h d", h=8),
                        scalar=1.0, op0=ALU.bypass, op1=ALU.add,
                        in1=bv_bc[:, ah * 512:(ah + 1) * 512].rearrange(
                            "p (h d) -> p h d", h=8))

            def attention(hp, qc):
                half, hpl = hp // 4, hp % 4
                QT, KT, V = QTh[half], KTh[half], Vh[half]
                pt = ptp.tile([P, S // P, 2, 512], BF16, tag="pt", name="pt")
                for kc in range(S // P):
                    st = psST.tile([P, 2, 512], F32, tag="st", name="st")
                    for e in range(2):
                        nc.tensor.matmul(
                            st[:, e],
                            KT[e * DEP:(e + 1) * DEP, hpl, kc * P:(kc + 1) * P],
                            QT[e * DEP:(e + 1) * DEP, hpl, qc * 512:(qc + 1) * 512],
                            start=True, stop=True)
                    nc.scalar.activation(
                        out=pt[:, kc], in_=st, func=AF.Exp,
                        bias=mask_bias[:, kc:kc + 1], scale=0.125)
                qsl = slice(qc * 512, (qc + 1) * 512)
                for e in range(2):
                    h = hp * 2 + e
                    cps = psCTX.tile([P, 512], F32, tag="cps", name="cps")
                    for kc in range(S // P):
                        nc.tensor.matmul(
                            cps[0:DEP + 1], V[:, kc, 2 * hpl + e, :],
                            pt[:, kc, e],
                            start=(kc == 0), stop=(kc == S // P - 1))
                    stg = pbm.tile([DEP + 1, 512], F32, tag="stg", name="stg")
                    nc.vector.tensor_copy(out=stg, in_=cps[0:DEP + 1])
                    # rows 0..63 -> ctxT (bf16, cast in DMA); row 64 -> denom
                    nc.gpsimd.dma_start(
                        out=ctxT[(h % 2) * DEP:(h % 2) * DEP + DEP, hp // 2, qsl],
                        in_=stg[0:DEP])
                    nc.sync.dma_start(out=denom[h:h + 1, qsl], in_=stg[DEP:DEP + 1])

            for ah in range(2):
                for th in range(2):
                    xt_h = pa.tile([P, D // P, TQ], BF16, tag="xt", name="xt")
                    nc.sync.dma_start(
                        out=xt_h, in_=xt_r[:, :, th * TQ:(th + 1) * TQ])
                    for m in range(4 * ah, 4 * ah + 4):
                        if th == 0:
                            project_qk(xt_h, 0, wq_r, bq_sb, QTh[ah], m)
                        project_qk(xt_h, th, wk_r, bk_sb, KTh[ah], m)
                    project_v(xt_h, th, ah)
                for hpl in range(4):
                    for qc in range(TQ // 512):
                        attention(4 * ah + hpl, qc)

            # softmax normalization: ctxT *= 1/denom (per head, per q)
            nc.vector.reciprocal_approx_fast(out=recip, in_=denom)
            nc.sync.dma_start(out=recip_dram.ap(), in_=recip)
            for h in range(H):
                for qc in range(TQ // 512):
                    rb = pbm.tile([P, 512], F32, tag="rb", name="rb")
                    prt = slice((h % 2) * DEP, (h % 2) * DEP + DEP)
                    qsl = slice(qc * 512, (qc + 1) * 512)
                    nc.sync.dma_start(
                        out=rb[prt],
                        in_=recip_dram.ap()[h:h + 1, qsl].partition_broadcast(
                            DEP).squeeze(1))
                    eng = nc.vector if (h + qc) % 2 == 0 else nc.gpsimd
                    eng.tensor_tensor(
                        out=ctxT[prt, h // 2, qsl], in0=ctxT[prt, h // 2, qsl],
                        in1=rb[prt], op=ALU.mult)

        qkv_pool.release()

        # ======== wo + residual + LN1 + transpose ========
        ffnin = tc.alloc_tile_pool(name="ffnin", bufs=1)
        out1 = ffnin.tile([P, TQ // P, D], F32)    # token qm*128+p
        out1T = ffnin.tile([P, D // P, TQ], F32R)  # d dm*128+p

        with tc.tile_pool(name="phC_c", bufs=1) as pcc, \
             tc.tile_pool(name="phC_s", bufs=4) as pcs, \
             tc.tile_pool(name="psC", bufs=2, space="PSUM") as psC, \
             tc.tile_pool(name="psCT", bufs=2, space="PSUM") as psCT:
            wo_sb = pcc.tile([P, D // P, D], BF16)
            wo_r = wo.ap().rearrange("(ko p) n -> p ko n", p=P)
            for ki in range(D // P):
                nc.sync.dma_start(out=wo_sb[:, ki], in_=wo_r[:, ki])
            g1_bc = pcc.tile([P, D], F32)
            be1_bc = pcc.tile([P, D], F32)
            nc.gpsimd.dma_start(out=g1_bc, in_=bcast(g1, D))
            nc.gpsimd.dma_start(out=be1_bc, in_=bcast(be1, D))
            xq_r = xq.ap().rearrange("(qm p) d -> p qm d", p=P)
            for qm in range(TQ // P):
                xq_t = pcs.tile([P, D], F32, tag="xqt", name="xqt")
                nc.sync.dma_start(out=xq_t, in_=xq_r[:, qm])
                pss = [psC.tile([P, 512], F32, tag=f"psC{n}", name=f"psC{n}")
                       for n in range(2)]
                for ki in range(D // P):
                    for n in range(2):
                        nc.tensor.matmul(
                            pss[n], ctxT[:, ki, qm * P:(qm + 1) * P],
                            wo_sb[:, ki, n * 512:(n + 1) * 512],
                            start=(ki == 0), stop=(ki == D // P - 1))
                for n in range(2):
                    nc.vector.scalar_tensor_tensor(
                        out=out1[:, qm, n * 512:(n + 1) * 512], in0=pss[n],
                        scalar=1.0, op0=ALU.bypass, op1=ALU.add,
                        in1=xq_t[:, n * 512:(n + 1) * 512])
                _layernorm(nc, pcs, out1[:, qm], g1_bc, be1_bc, eps_t)

            for qm in range(TQ // P):
                for dm in range(D // P):
                    tp = psCT.tile([P, P], F32, tag="tp", name="tp")
                    nc.tensor.transpose(tp, out1[:, qm, dm * P:(dm + 1) * P], ident)
                    nc.vector.tensor_copy(
                        out=out1T[:, dm, qm * P:(qm + 1) * P], in_=tp)

        ctx_pool.release()

        # ================= FFN + LN2 =================
        with tc.tile_pool(name="phD_c", bufs=1) as pdc, \
             tc.tile_pool(name="phD_h", bufs=1) as pdh, \
             tc.tile_pool(name="phD_w", bufs=4) as pdw, \
             tc.tile_pool(name="phD_s", bufs=4) as pds, \
             tc.tile_pool(name="psD1", bufs=3, space="PSUM") as psD1, \
             tc.tile_pool(name="psD2", bufs=1, space="PSUM") as psD2:
            g2_bc = pdc.tile([P, D], F32)
            be2_bc = pdc.tile([P, D], F32)
            nc.gpsimd.dma_start(out=g2_bc, in_=bcast(g2, D))
            nc.gpsimd.dma_start(out=be2_bc, in_=bcast(be2, D))
            w2_r = w2.ap().rearrange("(ko p) n -> p ko n", p=P)
            for qc in range(TQ // 512):
                hT = pdh.tile([P, DFF // P, 512], F32R, tag="hT", name="hT")
                for dm in range(DFF // P):
                    w1t = pdw.tile([P, D // P, P], F32R, tag="w1t", name="w1t")
                    nc.sync.dma_start(out=w1t, in_=w1p.ap()[dm])
                    ps = psD1.tile([P, 512], F32, tag="ps1", name="ps1")
                    for ki in range(D // P):
                        nc.tensor.matmul(
                            ps, w1t[:, ki],
                            out1T[:, ki, qc * 512:(qc + 1) * 512],
                            start=(ki == 0), stop=(ki == D // P - 1))
                    nc.scalar.activation(out=hT[:, dm], in_=ps, func=AF.Relu,
                                         bias=b1_sb[:, dm:dm + 1])
                for n in range(2):
                    pss = [psD2.tile([P, 512], F32, tag=f"ps2_{i}", name=f"ps2_{i}")
                           for i in range(4)]
                    for ds_ in range(DFF // P):
                        w2t = pdw.tile([P, 512], F32R, tag="w2t", name="w2t")
                        nc.sync.dma_start(
                            out=w2t, in_=w2_r[:, ds_, n * 512:(n + 1) * 512])
                        for j in range(4):
                            nc.tensor.matmul(
                                pss[j], hT[:, ds_, j * P:(j + 1) * P],
                                w2t, start=(ds_ == 0), stop=False)
                    for j in range(4):
                        nc.tensor.matmul(pss[j], ones_row,
                                         b2_sb[:, n * 512:(n + 1) * 512],
                                         start=False, stop=True)
                        qm = qc * 4 + j
                        nc.vector.scalar_tensor_tensor(
                            out=out1[:, qm, n * 512:(n + 1) * 512], in0=pss[j],
                            scalar=1.0, op0=ALU.bypass, op1=ALU.add,
                            in1=out1[:, qm, n * 512:(n + 1) * 512])
                for j in range(4):
                    qm = qc * 4 + j
                    _layernorm(nc, pds, out1[:, qm], g2_bc, be2_bc, eps_t)
                    nc.sync.dma_start(
                        out=out.ap().rearrange("(qm p) d -> p qm d", p=P)[:, qm],
                        in_=out1[:, qm])

        ffnin.release()
        consts.release()

    nc.compile()
    return nc


def _layernorm(nc, pool, x_ap, g_bc, be_bc, eps_t):
    """In-place LN over the free dim of x_ap [128, D]."""
    stats = pool.tile([P, D // 512, 6], F32, tag="ln_stats", name="ln_stats")
    mv = pool.tile([P, 2], F32, tag="ln_mv", name="ln_mv")
    xg = x_ap.rearrange("p (s f) -> p s f", f=512)
    for s in range(D // 512):
        nc.vector.bn_stats(out=stats[:, s], in_=xg[:, s])
    nc.vector.bn_aggr(out=mv, in_=stats)
    rstd = pool.tile([P, 1], F32, tag="ln_rstd", name="ln_rstd")
    nc.scalar.activation(out=rstd, in_=mv[:, 1:2], func=AF.Sqrt, bias=eps_t)
    nc.vector.reciprocal(out=rstd, in_=rstd)
    nc.vector.tensor_scalar(out=x_ap, in0=x_ap, scalar1=mv[:, 0:1],
                            scalar2=rstd, op0=ALU.subtract, op1=ALU.mult)
    nc.gpsimd.tensor_tensor(out=x_ap, in0=x_ap, in1=g_bc, op=ALU.mult)
    nc.gpsimd.tensor_tensor(out=x_ap, in0=x_ap, in1=be_bc, op=ALU.add)


_NC_CACHE = None


def _get_nc():
    global _NC_CACHE
    if _NC_CACHE is None:
        _NC_CACHE = build()
    return _NC_CACHE


def _prep_in_maps(inputs):
    x = np.asarray(inputs["x"], dtype=np.float32)        # [4, 2048, 1024]
    mask = np.asarray(inputs["mask"], dtype=np.float32)  # [4, 1, 1, 2048]
    bf = ml_dtypes.bfloat16
    w = {k: np.asarray(inputs[k], dtype=np.float32) for k in
         ("wq", "bq", "wk", "bk", "wv", "bv", "wo", "bo", "w1", "b1",
          "w2", "b2", "g1", "beta1", "g2", "beta2")}

    # w1 packed so each [128p, 8ko, 128n] tile is per-partition contiguous
    w1p = np.ascontiguousarray(
        w["w1"].reshape(D // P, P, DFF // P, P).transpose(2, 1, 0, 3))
    shared = {
        "wq": w["wq"].astype(bf), "wk": w["wk"].astype(bf), "wv": w["wv"].astype(bf),
        "bq": w["bq"], "bk": w["bk"], "bv": w["bv"],
        "wo": w["wo"].astype(bf), "w1p": w1p, "b1": w["b1"],
        "w2": w["w2"], "b2": w["b2"],
        "g1": w["g1"], "be1": w["beta1"], "g2": w["g2"], "be2": w["beta2"],
    }
    in_maps = []
    for c in range(8):
        b, half = c // 2, c % 2
        m = dict(shared)
        xb = x[b]
        order = np.r_[half * TQ:(half + 1) * TQ, (1 - half) * TQ:(2 - half) * TQ]
        m["xt"] = np.ascontiguousarray(xb[order].T).astype(bf)
        m["xq"] = np.ascontiguousarray(
            xb[half * TQ:(half + 1) * TQ] + w["bo"][None, :])
        m["maskb"] = np.ascontiguousarray(mask[b, 0, 0][order])
        in_maps.append(m)
    return in_maps


def kernel(**inputs):
    in_maps = _prep_in_maps(inputs)
    nc = _get_nc()
    res = run_bass_kernel_spmd(nc, in_maps, core_ids=list(range(8)))
    outp = np.empty((4, 2048, 1024), dtype=np.float32)
    for c in range(8):
        b, half = c // 2, c % 2
        outp[b, half * TQ:(half + 1) * TQ] = res.results[c]["out"]
    return outp


# revision 18
# speedup vs baseline: 1.1086x; 1.0074x over previous
"""Trainium2 Bass kernel for a transformer encoder layer (B=4, S=2048,
D=1024, H=16, DFF=4096, fp32).

Sharding: 8 cores = (batch b, query-half). Each core computes K/V for its
full batch (replicated within the pair) and Q/attention/FFN for its 1024
query tokens. No collectives. Host reorders tokens so each core's queries
are tokens 0..1023 of its (permutation-invariant) key set.

Per-core dataflow (layouts chosen so every matmul contracts along the
partition dim; the only on-chip transpose is out1 -> out1T):
  xT [D, S] host-transposed bf16
  QT/KT depth-major bf16 (head pairs packed 64+64 in partitions)
  V [tok, head, 65] bf16 with a ones column -> softmax denominator for free
  S^T = KT.T @ QT per (head, kc); exp on ACT (mask + 1/8 scale fused) -> PT bf16
  ctxT+denom = V_aug.T @ PT ; normalize via 1/denom broadcast ; ctxT bf16
  attn_out = ctxT.T @ wo (bf16) ; +x (+bo host-folded) ; LN1
  PE-transpose out1 -> out1T f32r ; FFN1 = relu(w1.T @ out1T + b1) -> hT f32r
  FFN2 = hT.T @ w2 (+b2 via K=1 ones-row matmul) ; +out1 ; LN2 -> out

Projection and attention emission is interleaved in two halves so the
scalar engine starts softmax exps while the PE is still projecting.
"""

import sys

sys.path.insert(0, "/opt/trn_rl_repo")

import numpy as np
import ml_dtypes

import concourse.bass as bass
import concourse.tile as tile
from concourse import bacc, mybir
from concourse.bass_utils import run_bass_kernel_spmd
from concourse.masks import make_identity

P = 128
D = 1024
S = 2048
TQ = 1024  # query tokens per core
H = 16
DEP = 64
DFF = 4096
F32 = mybir.dt.float32
F32R = mybir.dt.float32r
BF16 = mybir.dt.bfloat16
AF = mybir.ActivationFunctionType
ALU = mybir.AluOpType
EPS = 1e-6


def build():
    nc = bacc.Bacc("TRN2", target_bir_lowering=False)

    # ---- DRAM I/O ----
    xt = nc.dram_tensor("xt", [D, S], BF16, kind="ExternalInput")
    xq = nc.dram_tensor("xq", [TQ, D], F32, kind="ExternalInput")  # +bo folded
    maskb = nc.dram_tensor("maskb", [S], F32, kind="ExternalInput")
    wq = nc.dram_tensor("wq", [D, D], BF16, kind="ExternalInput")
    wk = nc.dram_tensor("wk", [D, D], BF16, kind="ExternalInput")
    wv = nc.dram_tensor("wv", [D, D], BF16, kind="ExternalInput")
    bq = nc.dram_tensor("bq", [D], F32, kind="ExternalInput")
    bk = nc.dram_tensor("bk", [D], F32, kind="ExternalInput")
    bv = nc.dram_tensor("bv", [D], F32, kind="ExternalInput")
    wo = nc.dram_tensor("wo", [D, D], BF16, kind="ExternalInput")
    w1p = nc.dram_tensor("w1p", [DFF // P, P, D // P, P], F32R, kind="ExternalInput")
    b1 = nc.dram_tensor("b1", [DFF], F32, kind="ExternalInput")
    w2 = nc.dram_tensor("w2", [DFF, D], F32R, kind="ExternalInput")
    b2 = nc.dram_tensor("b2", [D], F32R, kind="ExternalInput")
    g1 = nc.dram_tensor("g1", [D], F32, kind="ExternalInput")
    be1 = nc.dram_tensor("be1", [D], F32, kind="ExternalInput")
    g2 = nc.dram_tensor("g2", [D], F32, kind="ExternalInput")
    be2 = nc.dram_tensor("be2", [D], F32, kind="ExternalInput")
    out = nc.dram_tensor("out", [TQ, D], F32, kind="ExternalOutput")
    recip_dram = nc.dram_tensor("recip_scratch", [H, TQ], F32, kind="Internal")
    warm_dram = nc.dram_tensor("warm_scratch", [P, 16], F32, kind="Internal")

    def bcast(handle, n):
        return bass.AP(tensor=handle, offset=0, ap=[[0, P], [1, n]])

    with tile.TileContext(nc) as tc:
        consts = tc.alloc_tile_pool(name="consts", bufs=1)

        mask_bias = consts.tile([P, S // P], F32)  # mask[k]*-1e9, k = kc*128+p
        nc.gpsimd.dma_start(out=mask_bias,
                            in_=maskb.ap().rearrange("(kc p) -> p kc", p=P))
        nc.vector.tensor_scalar_mul(out=mask_bias, in0=mask_bias, scalar1=-1e9)

        bq_sb = consts.tile([P, D // P], F32)
        bk_sb = consts.tile([P, D // P], F32)
        nc.gpsimd.dma_start(out=bq_sb, in_=bq.ap().rearrange("(m p) -> p m", p=P))
        nc.gpsimd.dma_start(out=bk_sb, in_=bk.ap().rearrange("(m p) -> p m", p=P))
        bv_bc = consts.tile([P, D], F32)
        nc.gpsimd.dma_start(out=bv_bc, in_=bcast(bv, D))
        b1_sb = consts.tile([P, DFF // P], F32)
        nc.gpsimd.dma_start(out=b1_sb, in_=b1.ap().rearrange("(m p) -> p m", p=P))
        b2_sb = consts.tile([1, D], F32R)
        nc.gpsimd.dma_start(out=b2_sb, in_=bass.AP(tensor=b2, offset=0, ap=[[0, 1], [1, D]]))
        ones_row = consts.tile([1, P], F32R)
        ones_f32 = consts.tile([1, P], F32, tag="ones_f32")
        nc.vector.memset(ones_f32, 1.0)
        nc.vector.tensor_copy(out=ones_row, in_=ones_f32)
        ident = consts.tile([P, P], F32)
        make_identity(nc, ident)
        eps_t = consts.tile([P, 1], F32)
        nc.vector.memset(eps_t, EPS)

        # ---------- persistent activation tensors (split per half for
        # clean producer/consumer separation in the interleaved schedule) ----
        qkv_pool = tc.alloc_tile_pool(name="qkv", bufs=1)
        # depth-major Q/K: partition = (h%2)*64 + d, plane = h//2 - 4*half
        QTh = [qkv_pool.tile([P, 4, TQ], BF16, name=f"QT{i}") for i in range(2)]
        KTh = [qkv_pool.tile([P, 4, S], BF16, name=f"KT{i}") for i in range(2)]
        # V: [tok-part, tok-chunk, head - 8*half, 65]; col 64 = ones
        Vh = [qkv_pool.tile([P, S // P, 8, DEP + 1], BF16, name=f"V{i}")
              for i in range(2)]

        # ---------- attention output (right stack: outlives qkv) ----------
        ctx_pool = tc.alloc_tile_pool(name="ctx", bufs=1, side="right")
        ctxT = ctx_pool.tile([P, D // P, TQ], BF16)  # head h at part (h%2)*64
        denom = ctx_pool.tile([H, TQ], F32)
        recip = ctx_pool.tile([H, TQ], F32)

        # ---------- PE warm-up: a few fp32 matmuls so HAM un-throttles ----
        with tc.tile_pool(name="warm", bufs=1) as wp, \
             tc.tile_pool(name="warm_ps", bufs=1, space="PSUM") as wps:
            wjunk = wp.tile([P, 16], F32)
            wpt = wps.tile([P, P], F32)
            for i in range(10):
                nc.tensor.matmul(wpt, ident, ident, start=(i == 0),
                                 stop=(i == 9))
            nc.vector.tensor_copy(out=wjunk, in_=wpt[:, 0:16])
            nc.sync.dma_start(out=warm_dram.ap(), in_=wjunk)

        # ================= interleaved projections + attention ============
        with tc.tile_pool(name="phA", bufs=1) as pa, \
             tc.tile_pool(name="phA_wqk", bufs=3) as paw, \
             tc.tile_pool(name="phA_wv", bufs=1) as pawv, \
             tc.tile_pool(name="phB_pt", bufs=1) as ptp, \
             tc.tile_pool(name="phB_misc", bufs=2) as pbm, \
             tc.tile_pool(name="psA", bufs=2, space="PSUM") as psA, \
             tc.tile_pool(name="psB_st", bufs=2, space="PSUM") as psST, \
             tc.tile_pool(name="psB_ctx", bufs=2, space="PSUM") as psCTX:
            xt_r = xt.ap().rearrange("(ko p) t -> p ko t", p=P)
            for half in range(2):
                for t in range(S // P):
                    nc.gpsimd.memset(Vh[half][:, t, :, DEP:DEP + 1], 1.0)

            wq_r = wq.ap().rearrange("(ko p) n -> p ko n", p=P)
            wk_r = wk.ap().rearrange("(ko p) n -> p ko n", p=P)
            wv_r = wv.ap().rearrange("(ko p) n -> p ko n", p=P)

            def project_qk(xt_h, th, w_r, b_sb, Tt, m):
                # lhsT = w chunk [128din, 128dout]; rhs = xT token-half
                wt = paw.tile([P, D // P, P], BF16, tag="wqk", name="wqk")
                nc.sync.dma_start(out=wt, in_=w_r[:, :, m * P:(m + 1) * P])
                for qc in range(2):
                    ps = psA.tile([P, 512], F32, tag="psA", name="psA")
                    for ki in range(D // P):
                        nc.tensor.matmul(ps, wt[:, ki],
                                         xt_h[:, ki, qc * 512:(qc + 1) * 512],
                                         start=(ki == 0), stop=(ki == D // P - 1))
                    nc.vector.tensor_scalar(
                        out=Tt[:, m % 4, th * TQ + qc * 512:
                               th * TQ + (qc + 1) * 512], in0=ps,
                        scalar1=b_sb[:, m:m + 1], scalar2=None, op0=ALU.add)

            def project_v(xt_h, th, ah):
                wvt = pawv.tile([P, D // P, 512], BF16, tag="wv", name="wv")
                nc.sync.dma_start(out=wvt,
                                  in_=wv_r[:, :, ah * 512:(ah + 1) * 512])
                for tl in range(TQ // P):
                    t = th * (TQ // P) + tl
                    ps = psA.tile([P, 512], F32, tag="psA", name="psA")
                    for ki in range(D // P):
                        nc.tensor.matmul(ps, xt_h[:, ki, tl * P:(tl + 1) * P],
                                         wvt[:, ki],
                                         start=(ki == 0), stop=(ki == D // P - 1))
                    nc.vector.scalar_tensor_tensor(
                        out=Vh[ah][:, t, :, 0:DEP],
                        in0=ps.rearrange("p (h d) -> p h d", h=8),
                        scalar=1.0, op0=ALU.bypass, op1=ALU.add,
                        in1=bv_bc[:, ah * 512:(ah + 1) * 512].rearrange(
                            "p (h d) -> p h d", h=8))

            def attention(hp, qc):
                half, hpl = hp // 4, hp % 4
                QT, KT, V = QTh[half], KTh[half], Vh[half]
                pt = ptp.tile([P, S // P, 2, 512], BF16, tag="pt", name="pt")
                for kc in range(S // P):
                    st = psST.tile([P, 2, 512], F32, tag="st", name="st")
                    for e in range(2):
                        nc.tensor.matmul(
                            st[:, e],
                            KT[e * DEP:(e + 1) * DEP, hpl, kc * P:(kc + 1) * P],
                            QT[e * DEP:(e + 1) * DEP, hpl, qc * 512:(qc + 1) * 512],
                            start=True, stop=True)
                    nc.scalar.activation(
                        out=pt[:, kc], in_=st, func=AF.Exp,
                        bias=mask_bias[:, kc:kc + 1], scale=0.125)
                qsl = slice(qc * 512, (qc + 1) * 512)
                for e in range(2):
                    h = hp * 2 + e
                    cps = psCTX.tile([P, 512], F32, tag="cps", name="cps")
                    for kc in range(S // P):
                        nc.tensor.matmul(
                            cps[0:DEP + 1], V[:, kc, 2 * hpl + e, :],
                            pt[:, kc, e],
                            start=(kc == 0), stop=(kc == S // P - 1))
                    stg = pbm.tile([DEP + 1, 512], F32, tag="stg", name="stg")
                    nc.vector.tensor_copy(out=stg, in_=cps[0:DEP + 1])
                    # rows 0..63 -> ctxT (bf16, cast in DMA); row 64 -> denom
                    nc.gpsimd.dma_start(
                        out=ctxT[(h % 2) * DEP:(h % 2) * DEP + DEP, hp, qsl],
                        in_=stg[0:DEP])
                    nc.sync.dma_start(out=denom[h:h + 1, qsl], in_=stg[DEP:DEP + 1])

            for ah in range(2):
                for th in range(2):
                    xt_h = pa.tile([P, D // P, TQ], BF16, tag="xt", name="xt")
                    nc.sync.dma_start(
                        out=xt_h, in_=xt_r[:, :, th * TQ:(th + 1) * TQ])
                    for m in range(4 * ah, 4 * ah + 4):
                        if th == 0:
                            project_qk(xt_h, 0, wq_r, bq_sb, QTh[ah], m)
                        project_qk(xt_h, th, wk_r, bk_sb, KTh[ah], m)
                    project_v(xt_h, th, ah)
                for hpl in range(4):
                    for qc in range(TQ // 512):
                        attention(4 * ah + hpl, qc)

            # softmax normalization: ctxT *= 1/denom (per head, per q)
            nc.vector.reciprocal_approx_fast(out=recip, in_=denom)
            nc.sync.dma_start(out=recip_dram.ap(), in_=recip)
            for h in range(H):
                for qc in range(TQ // 512):
                    rb = pbm.tile([P, 512], F32, tag="rb", name="rb")
                    prt = slice((h % 2) * DEP, (h % 2) * DEP + DEP)
                    qsl = slice(qc * 512, (qc + 1) * 512)
                    nc.sync.dma_start(
                        out=rb[prt],
                        in_=recip_dram.ap()[h:h + 1, qsl].partition_broadcast(
                            DEP).squeeze(1))
                    eng = nc.vector if (h + qc) % 2 == 0 else nc.gpsimd
                    eng.tensor_tensor(
                        out=ctxT[prt, h // 2, qsl], in0=ctxT[prt, h // 2, qsl],
                        in1=rb[prt], op=ALU.mult)

        qkv_pool.release()

        # ======== wo + residual + LN1 + transpose ========
        ffnin = tc.alloc_tile_pool(name="ffnin", bufs=1)
        out1 = ffnin.tile([P, TQ // P, D], F32)    # token qm*128+p
        out1T = ffnin.tile([P, D // P, TQ], F32R)  # d dm*128+p

        with tc.tile_pool(name="phC_c", bufs=1) as pcc, \
             tc.tile_pool(name="phC_s", bufs=4) as pcs, \
             tc.tile_pool(name="psC", bufs=2, space="PSUM") as psC, \
             tc.tile_pool(name="psCT", bufs=2, space="PSUM") as psCT:
            wo_sb = pcc.tile([P, D // P, D], BF16)
            wo_r = wo.ap().rearrange("(ko p) n -> p ko n", p=P)
            for ki in range(D // P):
                nc.sync.dma_start(out=wo_sb[:, ki], in_=wo_r[:, ki])
            g1_bc = pcc.tile([P, D], F32)
            be1_bc = pcc.tile([P, D], F32)
            nc.gpsimd.dma_start(out=g1_bc, in_=bcast(g1, D))
            nc.gpsimd.dma_start(out=be1_bc, in_=bcast(be1, D))
            xq_r = xq.ap().rearrange("(qm p) d -> p qm d", p=P)
            for qm in range(TQ // P):
                xq_t = pcs.tile([P, D], F32, tag="xqt", name="xqt")
                nc.sync.dma_start(out=xq_t, in_=xq_r[:, qm])
                pss = [psC.tile([P, 512], F32, tag=f"psC{n}", name=f"psC{n}")
                       for n in range(2)]
                for ki in range(D // P):
                    for n in range(2):
                        nc.tensor.matmul(
                            pss[n], ctxT[:, ki, qm * P:(qm + 1) * P],
                            wo_sb[:, ki, n * 512:(n + 1) * 512],
                            start=(ki == 0), stop=(ki == D // P - 1))
                for n in range(2):
                    nc.vector.scalar_tensor_tensor(
                        out=out1[:, qm, n * 512:(n + 1) * 512], in0=pss[n],
                        scalar=1.0, op0=ALU.bypass, op1=ALU.add,
                        in1=xq_t[:, n * 512:(n + 1) * 512])
                _layernorm(nc, pcs, out1[:, qm], g1_bc, be1_bc, eps_t)

            for qm in range(TQ // P):
                for dm in range(D // P):
                    tp = psCT.tile([P, P], F32, tag="tp", name="tp")
                    nc.tensor.transpose(tp, out1[:, qm, dm * P:(dm + 1) * P], ident)
                    nc.vector.tensor_copy(
                        out=out1T[:, dm, qm * P:(qm + 1) * P], in_=tp)

        ctx_pool.release()

        # ================= FFN + LN2 =================
        with tc.tile_pool(name="phD_c", bufs=1) as pdc, \
             tc.tile_pool(name="phD_h", bufs=1) as pdh, \
             tc.tile_pool(name="phD_w", bufs=4) as pdw, \
             tc.tile_pool(name="phD_s", bufs=4) as pds, \
             tc.tile_pool(name="psD1", bufs=3, space="PSUM") as psD1, \
             tc.tile_pool(name="psD2", bufs=1, space="PSUM") as psD2:
            g2_bc = pdc.tile([P, D], F32)
            be2_bc = pdc.tile([P, D], F32)
            nc.gpsimd.dma_start(out=g2_bc, in_=bcast(g2, D))
            nc.gpsimd.dma_start(out=be2_bc, in_=bcast(be2, D))
            w2_r = w2.ap().rearrange("(ko p) n -> p ko n", p=P)
            for qc in range(TQ // 512):
                hT = pdh.tile([P, DFF // P, 512], F32R, tag="hT", name="hT")
                for dm in range(DFF // P):
                    w1t = pdw.tile([P, D // P, P], F32R, tag="w1t", name="w1t")
                    nc.sync.dma_start(out=w1t, in_=w1p.ap()[dm])
                    ps = psD1.tile([P, 512], F32, tag="ps1", name="ps1")
                    for ki in range(D // P):
                        nc.tensor.matmul(
                            ps, w1t[:, ki],
                            out1T[:, ki, qc * 512:(qc + 1) * 512],
                            start=(ki == 0), stop=(ki == D // P - 1))
                    nc.scalar.activation(out=hT[:, dm], in_=ps, func=AF.Relu,
                                         bias=b1_sb[:, dm:dm + 1])
                for n in range(2):
                    pss = [psD2.tile([P, 512], F32, tag=f"ps2_{i}", name=f"ps2_{i}")
                           for i in range(4)]
                    for ds_ in range(DFF // P):
                        w2t = pdw.tile([P, 512], F32R, tag="w2t", name="w2t")
                        nc.sync.dma_start(
                            out=w2t, in_=w2_r[:, ds_, n * 512:(n + 1) * 512])
                        for j in range(4):
                            nc.tensor.matmul(
                                pss[j], hT[:, ds_, j * P:(j + 1) * P],
                                w2t, start=(ds_ == 0), stop=False)
                    for j in range(4):
                        nc.tensor.matmul(pss[j], ones_row,
                                         b2_sb[:, n * 512:(n + 1) * 512],
                                         start=False, stop=True)
                        qm = qc * 4 + j
                        nc.vector.scalar_tensor_tensor(
                            out=out1[:, qm, n * 512:(n + 1) * 512], in0=pss[j],
                            scalar=1.0, op0=ALU.bypass, op1=ALU.add,
                            in1=out1[:, qm, n * 512:(n + 1) * 512])
                for j in range(4):
                    qm = qc * 4 + j
                    _layernorm(nc, pds, out1[:, qm], g2_bc, be2_bc, eps_t)
                    nc.sync.dma_start(
                        out=out.ap().rearrange("(qm p) d -> p qm d", p=P)[:, qm],
                        in_=out1[:, qm])

        ffnin.release()
        consts.release()

    nc.compile()
    return nc


def _layernorm(nc, pool, x_ap, g_bc, be_bc, eps_t):
    """In-place LN over the free dim of x_ap [128, D]."""
    stats = pool.tile([P, D // 512, 6], F32, tag="ln_stats", name="ln_stats")
    mv = pool.tile([P, 2], F32, tag="ln_mv", name="ln_mv")
    xg = x_ap.rearrange("p (s f) -> p s f", f=512)
    for s in range(D // 512):
        nc.vector.bn_stats(out=stats[:, s], in_=xg[:, s])
    nc.vector.bn_aggr(out=mv, in_=stats)
    rstd = pool.tile([P, 1], F32, tag="ln_rstd", name="ln_rstd")
    nc.scalar.activation(out=rstd, in_=mv[:, 1:2], func=AF.Sqrt, bias=eps_t)
    nc.vector.reciprocal(out=rstd, in_=rstd)
    nc.vector.tensor_scalar(out=x_ap, in0=x_ap, scalar1=mv[:, 0:1],
                            scalar2=rstd, op0=ALU.subtract, op1=ALU.mult)
    nc.gpsimd.tensor_tensor(out=x_ap, in0=x_ap, in1=g_bc, op=ALU.mult)
    nc.gpsimd.tensor_tensor(out=x_ap, in0=x_ap, in1=be_bc, op=ALU.add)


_NC_CACHE = None


def _get_nc():
    global _NC_CACHE
    if _NC_CACHE is None:
        _NC_CACHE = build()
    return _NC_CACHE


def _prep_in_maps(inputs):
    x = np.asarray(inputs["x"], dtype=np.float32)        # [4, 2048, 1024]
    mask = np.asarray(inputs["mask"], dtype=np.float32)  # [4, 1, 1, 2048]
    bf = ml_dtypes.bfloat16
    w = {k: np.asarray(inputs[k], dtype=np.float32) for k in
         ("wq", "bq", "wk", "bk", "wv", "bv", "wo", "bo", "w1", "b1",
          "w2", "b2", "g1", "beta1", "g2", "beta2")}

    # w1 packed so each [128p, 8ko, 128n] tile is per-partition contiguous
    w1p = np.ascontiguousarray(
        w["w1"].reshape(D // P, P, DFF // P, P).transpose(2, 1, 0, 3))
    shared = {
        "wq": w["wq"].astype(bf), "wk": w["wk"].astype(bf), "wv": w["wv"].astype(bf),
        "bq": w["bq"], "bk": w["bk"], "bv": w["bv"],
        "wo": w["wo"].astype(bf), "w1p": w1p, "b1": w["b1"],
        "w2": w["w2"], "b2": w["b2"],
        "g1": w["g1"], "be1": w["beta1"], "g2": w["g2"], "be2": w["beta2"],
    }
    in_maps = []
    for c in range(8):
        b, half = c // 2, c % 2
        m = dict(shared)
        xb = x[b]
        order = np.r_[half * TQ:(half + 1) * TQ, (1 - half) * TQ:(2 - half) * TQ]
        m["xt"] = np.ascontiguousarray(xb[order].T).astype(bf)
        m["xq"] = np.ascontiguousarray(
            xb[half * TQ:(half + 1) * TQ] + w["bo"][None, :])
        m["maskb"] = np.ascontiguousarray(mask[b, 0, 0][order])
        in_maps.append(m)
    return in_maps


def kernel(**inputs):
    in_maps = _prep_in_maps(inputs)
    nc = _get_nc()
    res = run_bass_kernel_spmd(nc, in_maps, core_ids=list(range(8)))
    outp = np.empty((4, 2048, 1024), dtype=np.float32)
    for c in range(8):
        b, half = c // 2, c % 2
        outp[b, half * TQ:(half + 1) * TQ] = res.results[c]["out"]
    return outp
